# revision 23
# baseline (speedup 1.0000x reference)
"""Trainium2 Bass kernel for nn_BondLengthDeviation.

Computes, for T*B=8 (timestep, replica) frames of N=512 atoms, the periodic
minimum-image length of 768 bonds, plus the per-replica max |deviation| from
mean bond lengths.

Key formulation: the full [N,N] distance matrix is never needed — only the 768
bond pairs.  Per bond k: d0 = r[i_k] - r[j_k] and the reference's min over the
27 periodic images of |d0 - shift|.

Sharding: bonds are split across the 8 cores (96 bonds/core); every core
processes all 8 (t, b) frames for its bond chunk.  The bond-endpoint gather
runs on-device as one-hot matmuls on the PE array with *signed* one-hots
(+1 at atom i, -1 at atom j, built from the bond-index inputs with
iota + is_equal compares), so d0 = S^T @ coords in 4 accumulating matmuls.

Two variants, chosen on host by inspecting `cell`:
  * diag (cell == box*I, which setup_inputs always produces): the 27-image
    min decomposes per component into the squared wrap distance
    min(d0^2, (d0-box)^2, (d0+box)^2)  (|d0| < box always holds here).
  * general: -2*d0.shift + |shift|^2 via a PE transpose of d0 and one matmul
    against a host-prepared block-diagonal shift table (a pure function of
    `cell`, a tiny replicated constant per the sharding hint), then a
    free-axis min-reduce and |d0|^2.

Per the sharding hint, no cross-device communication is needed until the
final max over T: the deviation max is folded in at gather/unshard time on
host together with the bond_lens assembly.
"""

import os
import sys

import numpy as np


def _ensure_import_path():
    try:
        import concourse  # noqa: F401
        return
    except ImportError:
        pass
    for p in ("/opt/trn_rl_repo", "/root/.axon_site/_ro/trn_rl_repo"):
        if os.path.isdir(os.path.join(p, "concourse")):
            sys.path.insert(0, p)
            return
    raise ImportError("cannot locate the concourse (bass) package")


# Problem shapes (hardcoded per the harness contract).
T, B, N = 2, 4, 512
TB = T * B            # 8 frames
NBONDS = 768
NCORES = 8
BPC = NBONDS // NCORES  # 96 bonds per core
A = N // 128            # 4 atom partition-tiles

_NC_CACHE = {}
LAST_RESULTS = None  # BassKernelResults of the most recent run (for test.py)


def _combos_np():
    # Mirrors reference._combos: the 27 periodic image shift multipliers.
    pos = np.arange(-1, 2)
    g = np.stack(np.meshgrid(pos, pos, pos, indexing="xy"), axis=0)
    return g.transpose(3, 2, 1, 0).reshape(-1, 3).astype(np.float32)  # [27,3]


def _common_prolog(nc, sb, mybir):
    """Input DMAs (spread over two queues) shared by both variants."""
    f32 = mybir.dt.float32
    f16 = mybir.dt.float16
    bf16 = mybir.dt.bfloat16

    # coords as bf16 (hi, lo) pairs: hi + lo reconstructs fp32 coords to
    # ~2^-18 relative, and bf16 matmuls run single-pass on the PE (fp32
    # matmuls are double-pumped with two weight loads).
    coords = nc.dram_tensor(
        "coords", [128, A, TB, 3, 2], bf16, kind="ExternalInput"
    )
    # bond ids replicated to all partitions on host (sharding prep)
    bb_d = nc.dram_tensor("bb", [128, 2, BPC], f16, kind="ExternalInput")

    bb_sb = sb.tile([128, 2, BPC], f16)
    nc.sync.dma_start(bb_sb[:], bb_d[:])
    cs = sb.tile([128, A, TB, 3, 2], bf16)
    nc.sync.dma_start(cs[0:64], coords[0:64])
    nc.scalar.dma_start(cs[64:128], coords[64:128])

    # Dummy early Sqrt so bacc's lazy ACT-table-load for the sqrt table runs
    # during the (otherwise idle) prolog instead of on the critical path
    # between the Square activations and the real sqrt.
    dummy = sb.tile([1, 1], f32)
    nc.vector.memset(dummy[:], 1.0)
    dummy2 = sb.tile([1, 1], f32)
    nc.scalar.sqrt(dummy2[:], dummy[:])
    return cs, bb_sb


def _emit_gather(nc, sb, ps, cs, bb_sb, mybir):
    """Build signed one-hots from replicated bond ids, gather d0 into PSUM."""
    f32 = mybir.dt.float32
    f16 = mybir.dt.float16
    AL = mybir.AluOpType

    iotas = []
    for a in range(A):
        it = sb.tile([128, 1], f32, tag=f"iota{a}")
        nc.gpsimd.iota(
            it[:],
            pattern=[[0, 1]],
            base=128 * a,
            channel_multiplier=1,
            allow_small_or_imprecise_dtypes=True,
        )
        iotas.append(it)

    bf16 = mybir.dt.bfloat16
    ohs = sb.tile([128, A, BPC], bf16)
    for a in range(A):
        ohij = sb.tile([128, 2, BPC], f16, tag=f"ohij{a}")
        nc.vector.tensor_scalar(ohij[:], bb_sb[:], iotas[a][:], None, AL.is_equal)
        nc.vector.tensor_sub(ohs[:, a, :], ohij[:, 0, :], ohij[:, 1, :])

    # gather hi and lo halves side by side, then d0 = hi + lo in fp32
    psum_hl = ps.tile([BPC, TB, 3, 2], f32)
    for a in range(A):
        nc.tensor.matmul(
            psum_hl[:], ohs[:, a, :], cs[:, a, :, :, :],
            start=(a == 0), stop=(a == A - 1),
        )
    hl_sb = sb.tile([BPC, TB, 3, 2], f32)
    nc.vector.tensor_copy(hl_sb[:], psum_hl[:])
    d0 = sb.tile([BPC, TB, 3], f32)
    nc.vector.tensor_tensor(
        d0[:], hl_sb[:, :, :, 0], hl_sb[:, :, :, 1], AL.add
    )
    return d0


def _build_nc_diag(box: float):
    """Fast path for cell == box * I: per-component squared wrap distance."""
    _ensure_import_path()
    import concourse.bacc as bacc
    import concourse.mybir as mybir
    import concourse.tile as tile

    f32 = mybir.dt.float32
    AL = mybir.AluOpType
    AX = mybir.AxisListType
    AF = mybir.ActivationFunctionType

    nc = bacc.Bacc("TRN2", target_bir_lowering=False, debug=False)
    out_bl = nc.dram_tensor("out_bl", [BPC, TB], f32, kind="ExternalOutput")

    with tile.TileContext(nc) as tc:
        with (
            tc.tile_pool(name="sb", bufs=1) as sb,
            tc.tile_pool(name="ps", bufs=1, space="PSUM") as ps,
        ):
            cs, bb_sb = _common_prolog(nc, sb, mybir)
            d0 = _emit_gather(nc, sb, ps, cs, bb_sb, mybir)

            # squared wrap distance per component:
            # min(d0^2, (d0-box)^2, (d0+box)^2), |d0| < box.
            bias_p = sb.tile([BPC, 1], f32)
            nc.gpsimd.memset(bias_p[:], box)
            bias_m = sb.tile([BPC, 1], f32)
            nc.gpsimd.memset(bias_m[:], -box)
            s1 = sb.tile([BPC, TB, 3], f32)
            nc.scalar.activation(s1[:], d0[:], AF.Square, bias=bias_p[:])
            s2 = sb.tile([BPC, TB, 3], f32)
            nc.scalar.activation(s2[:], d0[:], AF.Square, bias=bias_m[:])
            s0 = sb.tile([BPC, TB, 3], f32)
            nc.vector.tensor_tensor(s0[:], d0[:], d0[:], AL.mult)
            mn = sb.tile([BPC, TB, 3], f32)
            nc.vector.tensor_tensor(mn[:], s1[:], s2[:], AL.min)
            mn2 = sb.tile([BPC, TB, 3], f32)
            nc.vector.tensor_tensor(mn2[:], s0[:], mn[:], AL.min)
            bl2 = sb.tile([BPC, TB], f32)
            nc.vector.tensor_reduce(bl2[:], mn2[:], axis=AX.X, op=AL.add)
            bl = sb.tile([BPC, TB], f32)
            nc.scalar.sqrt(bl[:], bl2[:])
            nc.sync.dma_start(out_bl[:], bl[:])

    nc.compile()
    return nc


def _build_nc_general():
    """General cell: block-diagonal shift table + PE transpose + min-reduce."""
    _ensure_import_path()
    import concourse.bacc as bacc
    import concourse.mybir as mybir
    import concourse.tile as tile
    from concourse.masks import make_identity

    f32 = mybir.dt.float32
    AL = mybir.AluOpType
    AX = mybir.AxisListType

    nc = bacc.Bacc("TRN2", target_bir_lowering=False, debug=False)
    bd_d = nc.dram_tensor("bd", [TB * 3 + 1, TB, 27], f32, kind="ExternalInput")
    out_bl = nc.dram_tensor("out_bl", [BPC, TB], f32, kind="ExternalOutput")

    with tile.TileContext(nc) as tc:
        with (
            tc.tile_pool(name="sb", bufs=1) as sb,
            tc.tile_pool(name="ps", bufs=1, space="PSUM") as ps,
        ):
            cs, bb_sb = _common_prolog(nc, sb, mybir)
            bd_sb = sb.tile([TB * 3 + 1, TB, 27], f32)
            nc.sync.dma_start(bd_sb[:], bd_d[:])
            d0 = _emit_gather(nc, sb, ps, cs, bb_sb, mybir)

            # d0 plus a ones column, then PE-transpose to [(tb,c)+1, bond]
            d0_sb = sb.tile([BPC, TB * 3 + 1], f32)
            nc.vector.tensor_copy(
                d0_sb[:, 0 : TB * 3].rearrange("p (t c) -> p t c", c=3), d0[:]
            )
            nc.vector.memset(d0_sb[:, TB * 3 : TB * 3 + 1], 1.0)

            # |d0|^2 per (bond, frame)
            d0_v = d0_sb[:, 0 : TB * 3].rearrange("p (t c) -> p t c", c=3)
            sq = sb.tile([BPC, TB, 3], f32)
            nc.vector.tensor_tensor(sq[:], d0_v, d0_v, AL.mult)
            d0sq = sb.tile([BPC, TB], f32)
            nc.vector.tensor_reduce(d0sq[:], sq[:], axis=AX.X, op=AL.add)

            ident = sb.tile([BPC, BPC], f32)
            make_identity(nc, ident[:])
            psum_T = ps.tile([TB * 3 + 1, BPC], f32)
            nc.tensor.transpose(psum_T[:], d0_sb[:], ident[:])
            T_sb = sb.tile([TB * 3 + 1, BPC], f32)
            nc.vector.tensor_copy(T_sb[:], psum_T[:])

            # G[k, tb, s] = -2 d0.shift + |shift|^2 (one matmul, N=216)
            psum_G = ps.tile([BPC, TB, 27], f32)
            nc.tensor.matmul(psum_G[:], T_sb[:], bd_sb[:], start=True, stop=True)

            Gmin = sb.tile([BPC, TB], f32)
            nc.vector.tensor_reduce(Gmin[:], psum_G[:], axis=AX.X, op=AL.min)
            bl2 = sb.tile([BPC, TB], f32)
            nc.vector.tensor_tensor(bl2[:], Gmin[:], d0sq[:], AL.add)
            bl2c = sb.tile([BPC, TB], f32)
            nc.vector.tensor_scalar_max(bl2c[:], bl2[:], 0.0)
            bl = sb.tile([BPC, TB], f32)
            nc.scalar.sqrt(bl[:], bl2c[:])
            nc.sync.dma_start(out_bl[:], bl[:])

    nc.compile()
    return nc


def _diag_box(cell):
    """Return box if cell == box*I (the layout setup_inputs produces), else None."""
    d = np.diag(cell)
    if (
        d[0] > 0
        and np.all(np.abs(d - d[0]) <= 1e-6 * abs(d[0]))
        and np.all(np.abs(cell - np.diag(d)) <= 1e-6 * abs(d[0]))
    ):
        return float(d[0])
    return None


def _get_nc(cell):
    box = _diag_box(cell)
    key = ("diag", box) if box is not None else ("general",)
    if key not in _NC_CACHE:
        _NC_CACHE[key] = (
            _build_nc_diag(box) if box is not None else _build_nc_general()
        )
    return _NC_CACHE[key], box


def _make_in_maps(radii, cell, bonds, need_bd):
    """Host-side sharding: per-core input dicts for the SPMD run."""
    import ml_dtypes

    bf16 = ml_dtypes.bfloat16
    arr = radii.reshape(TB, A, 128, 3).transpose(2, 1, 0, 3)  # [p, a, tb, c]
    hi = arr.astype(bf16)
    lo = (arr - hi.astype(np.float32)).astype(bf16)
    cs = np.ascontiguousarray(np.stack([hi, lo], axis=-1))  # [p, a, tb, c, 2]

    bd = None
    if need_bd:
        # Shift table from the tiny replicated constant `cell`.
        shifts = _combos_np() @ cell                      # [27, 3]
        ssq = (shifts * shifts).sum(axis=1)               # [27]
        bd = np.zeros((TB * 3 + 1, TB, 27), np.float32)
        for t in range(TB):
            bd[3 * t : 3 * t + 3, t, :] = -2.0 * shifts.T
        bd[TB * 3, :, :] = ssq[None, :]
        bd = np.ascontiguousarray(bd)

    in_maps = []
    for c in range(NCORES):
        sl = slice(c * BPC, (c + 1) * BPC)
        bb = np.broadcast_to(
            bonds[sl].T.astype(np.float16).reshape(1, 2, BPC), (128, 2, BPC)
        )
        m = {"coords": cs, "bb": np.ascontiguousarray(bb)}
        if need_bd:
            m["bd"] = bd
        in_maps.append(m)
    return in_maps


def kernel(**inputs):
    global LAST_RESULTS
    _ensure_import_path()
    from concourse.bass_utils import run_bass_kernel_spmd

    radii = np.asarray(inputs["stacked_radii"], dtype=np.float32)  # [T,B,N,3]
    cell = np.asarray(inputs["cell"], dtype=np.float32)
    bonds = np.asarray(inputs["bonds"]).astype(np.int64)           # [768,2]
    mbl = np.asarray(inputs["mean_bond_lens"], dtype=np.float32)   # [768]

    nc, box = _get_nc(cell)
    in_maps = _make_in_maps(radii, cell, bonds, need_bd=box is None)
    res = run_bass_kernel_spmd(nc, in_maps, core_ids=list(range(NCORES)))
    LAST_RESULTS = res

    # gather/unshard: assemble bond_lens; fold the deviation max over
    # (bonds, T) per replica at the same time (the hint's "final max over T").
    bond_lens = np.empty((T, B, NBONDS), np.float32)
    for c in range(NCORES):
        bond_lens[:, :, c * BPC : (c + 1) * BPC] = (
            res.results[c]["out_bl"].T.reshape(T, B, BPC)
        )
    dev = np.abs(bond_lens - mbl[None, None, :])
    max_dev = dev.max(axis=-1).max(axis=0).astype(np.float32)  # [B]
    return bond_lens, max_dev


# revision 25
# speedup vs baseline: 1.0120x; 1.0120x over previous
"""Trainium2 Bass kernel for nn_BondLengthDeviation.

Computes, for T*B=8 (timestep, replica) frames of N=512 atoms, the periodic
minimum-image length of 768 bonds, plus the per-replica max |deviation| from
mean bond lengths.

Key formulation: the full [N,N] distance matrix is never needed — only the 768
bond pairs.  Per bond k: d0 = r[i_k] - r[j_k] and the reference's min over the
27 periodic images of |d0 - shift|.

Sharding: bonds are split across the 8 cores (96 bonds/core); every core
processes all 8 (t, b) frames for its bond chunk.  The bond-endpoint gather
runs on-device as one-hot matmuls on the PE array with *signed* one-hots
(+1 at atom i, -1 at atom j, built from the bond-index inputs with
iota + is_equal compares), so d0 = S^T @ coords in 4 accumulating matmuls.

Two variants, chosen on host by inspecting `cell`:
  * diag (cell == box*I, which setup_inputs always produces): the 27-image
    min decomposes per component into the squared wrap distance
    min(d0^2, (d0-box)^2, (d0+box)^2)  (|d0| < box always holds here).
  * general: -2*d0.shift + |shift|^2 via a PE transpose of d0 and one matmul
    against a host-prepared block-diagonal shift table (a pure function of
    `cell`, a tiny replicated constant per the sharding hint), then a
    free-axis min-reduce and |d0|^2.

Per the sharding hint, no cross-device communication is needed until the
final max over T: the deviation max is folded in at gather/unshard time on
host together with the bond_lens assembly.
"""

import os
import sys

import numpy as np


def _ensure_import_path():
    try:
        import concourse  # noqa: F401
        return
    except ImportError:
        pass
    for p in ("/opt/trn_rl_repo", "/root/.axon_site/_ro/trn_rl_repo"):
        if os.path.isdir(os.path.join(p, "concourse")):
            sys.path.insert(0, p)
            return
    raise ImportError("cannot locate the concourse (bass) package")


# Problem shapes (hardcoded per the harness contract).
T, B, N = 2, 4, 512
TB = T * B            # 8 frames
NBONDS = 768
NCORES = 8
BPC = NBONDS // NCORES  # 96 bonds per core
A = N // 128            # 4 atom partition-tiles

_NC_CACHE = {}
LAST_RESULTS = None  # BassKernelResults of the most recent run (for test.py)


def _combos_np():
    # Mirrors reference._combos: the 27 periodic image shift multipliers.
    pos = np.arange(-1, 2)
    g = np.stack(np.meshgrid(pos, pos, pos, indexing="xy"), axis=0)
    return g.transpose(3, 2, 1, 0).reshape(-1, 3).astype(np.float32)  # [27,3]


def _common_prolog(nc, sb, mybir):
    """Input DMAs (spread over two queues) shared by both variants."""
    f32 = mybir.dt.float32
    f16 = mybir.dt.float16
    bf16 = mybir.dt.bfloat16

    # coords as bf16 (hi, lo) pairs: hi + lo reconstructs fp32 coords to
    # ~2^-18 relative, and bf16 matmuls run single-pass on the PE (fp32
    # matmuls are double-pumped with two weight loads).
    coords = nc.dram_tensor(
        "coords", [128, A, TB, 3, 2], bf16, kind="ExternalInput"
    )
    # bond ids replicated to all partitions on host (sharding prep)
    bb_d = nc.dram_tensor("bb", [128, 2, BPC], f16, kind="ExternalInput")

    bb_sb = sb.tile([128, 2, BPC], f16)
    nc.sync.dma_start(bb_sb[:], bb_d[:])
    cs = sb.tile([128, A, TB, 3, 2], bf16)
    nc.sync.dma_start(cs[0:64], coords[0:64])
    nc.scalar.dma_start(cs[64:128], coords[64:128])

    # Dummy early Sqrt so bacc's lazy ACT-table-load for the sqrt table runs
    # during the (otherwise idle) prolog instead of on the critical path
    # between the Square activations and the real sqrt.
    dummy = sb.tile([1, 1], f32)
    nc.vector.memset(dummy[:], 1.0)
    dummy2 = sb.tile([1, 1], f32)
    nc.scalar.sqrt(dummy2[:], dummy[:])
    return cs, bb_sb


def _emit_gather(nc, sb, ps, cs, bb_sb, mybir):
    """Build signed one-hots from replicated bond ids, gather d0 into PSUM."""
    f32 = mybir.dt.float32
    f16 = mybir.dt.float16
    AL = mybir.AluOpType

    iotas = []
    for a in range(A):
        it = sb.tile([128, 1], f32, tag=f"iota{a}")
        nc.gpsimd.iota(
            it[:],
            pattern=[[0, 1]],
            base=128 * a,
            channel_multiplier=1,
            allow_small_or_imprecise_dtypes=True,
        )
        iotas.append(it)

    bf16 = mybir.dt.bfloat16
    ohs = sb.tile([128, A, BPC], bf16)
    for a in range(A):
        ohij = sb.tile([128, 2, BPC], f16, tag=f"ohij{a}")
        nc.vector.tensor_scalar(ohij[:], bb_sb[:], iotas[a][:], None, AL.is_equal)
        nc.vector.tensor_sub(ohs[:, a, :], ohij[:, 0, :], ohij[:, 1, :])

    # gather hi and lo halves side by side, then d0 = hi + lo in fp32
    psum_hl = ps.tile([BPC, TB, 3, 2], f32)
    for a in range(A):
        nc.tensor.matmul(
            psum_hl[:], ohs[:, a, :], cs[:, a, :, :, :],
            start=(a == 0), stop=(a == A - 1),
        )
    dh = sb.tile([BPC, TB, 3], f32)
    nc.vector.tensor_scalar_add(dh[:], psum_hl[:, :, :, 0], 0.0)
    d0 = sb.tile([BPC, TB, 3], f32)
    nc.vector.tensor_tensor(d0[:], dh[:], psum_hl[:, :, :, 1], AL.add)
    return d0


def _build_nc_diag(box: float):
    """Fast path for cell == box * I: per-component squared wrap distance."""
    _ensure_import_path()
    import concourse.bacc as bacc
    import concourse.mybir as mybir
    import concourse.tile as tile

    f32 = mybir.dt.float32
    AL = mybir.AluOpType
    AX = mybir.AxisListType
    AF = mybir.ActivationFunctionType

    nc = bacc.Bacc("TRN2", target_bir_lowering=False, debug=False)
    out_bl = nc.dram_tensor("out_bl", [BPC, TB], f32, kind="ExternalOutput")

    with tile.TileContext(nc) as tc:
        with (
            tc.tile_pool(name="sb", bufs=1) as sb,
            tc.tile_pool(name="ps", bufs=1, space="PSUM") as ps,
        ):
            cs, bb_sb = _common_prolog(nc, sb, mybir)
            d0 = _emit_gather(nc, sb, ps, cs, bb_sb, mybir)

            # squared wrap distance per component:
            # min(d0^2, (d0-box)^2, (d0+box)^2), |d0| < box.
            # s1 on ACT (runs concurrently with the DVE chain), rest on DVE.
            bias_p = sb.tile([BPC, 1], f32)
            nc.gpsimd.memset(bias_p[:], box)
            s1 = sb.tile([BPC, TB, 3], f32)
            nc.scalar.activation(s1[:], d0[:], AF.Square, bias=bias_p[:])
            c2 = sb.tile([BPC, TB, 3], f32)
            nc.vector.tensor_scalar(c2[:], d0[:], box, None, AL.subtract)
            s2 = sb.tile([BPC, TB, 3], f32)
            nc.vector.tensor_tensor(s2[:], c2[:], c2[:], AL.mult)
            s0 = sb.tile([BPC, TB, 3], f32)
            nc.vector.tensor_tensor(s0[:], d0[:], d0[:], AL.mult)
            mn = sb.tile([BPC, TB, 3], f32)
            nc.vector.tensor_tensor(mn[:], s0[:], s2[:], AL.min)
            mn2 = sb.tile([BPC, TB, 3], f32)
            nc.vector.tensor_tensor(mn2[:], mn[:], s1[:], AL.min)
            bl2 = sb.tile([BPC, TB], f32)
            nc.vector.tensor_reduce(bl2[:], mn2[:], axis=AX.X, op=AL.add)
            bl = sb.tile([BPC, TB], f32)
            nc.scalar.sqrt(bl[:], bl2[:])
            nc.sync.dma_start(out_bl[:], bl[:])

    nc.compile()
    return nc


def _build_nc_general():
    """General cell: block-diagonal shift table + PE transpose + min-reduce."""
    _ensure_import_path()
    import concourse.bacc as bacc
    import concourse.mybir as mybir
    import concourse.tile as tile
    from concourse.masks import make_identity

    f32 = mybir.dt.float32
    AL = mybir.AluOpType
    AX = mybir.AxisListType

    nc = bacc.Bacc("TRN2", target_bir_lowering=False, debug=False)
    bd_d = nc.dram_tensor("bd", [TB * 3 + 1, TB, 27], f32, kind="ExternalInput")
    out_bl = nc.dram_tensor("out_bl", [BPC, TB], f32, kind="ExternalOutput")

    with tile.TileContext(nc) as tc:
        with (
            tc.tile_pool(name="sb", bufs=1) as sb,
            tc.tile_pool(name="ps", bufs=1, space="PSUM") as ps,
        ):
            cs, bb_sb = _common_prolog(nc, sb, mybir)
            bd_sb = sb.tile([TB * 3 + 1, TB, 27], f32)
            nc.sync.dma_start(bd_sb[:], bd_d[:])
            d0 = _emit_gather(nc, sb, ps, cs, bb_sb, mybir)

            # d0 plus a ones column, then PE-transpose to [(tb,c)+1, bond]
            d0_sb = sb.tile([BPC, TB * 3 + 1], f32)
            nc.vector.tensor_copy(
                d0_sb[:, 0 : TB * 3].rearrange("p (t c) -> p t c", c=3), d0[:]
            )
            nc.vector.memset(d0_sb[:, TB * 3 : TB * 3 + 1], 1.0)

            # |d0|^2 per (bond, frame)
            d0_v = d0_sb[:, 0 : TB * 3].rearrange("p (t c) -> p t c", c=3)
            sq = sb.tile([BPC, TB, 3], f32)
            nc.vector.tensor_tensor(sq[:], d0_v, d0_v, AL.mult)
            d0sq = sb.tile([BPC, TB], f32)
            nc.vector.tensor_reduce(d0sq[:], sq[:], axis=AX.X, op=AL.add)

            ident = sb.tile([BPC, BPC], f32)
            make_identity(nc, ident[:])
            psum_T = ps.tile([TB * 3 + 1, BPC], f32)
            nc.tensor.transpose(psum_T[:], d0_sb[:], ident[:])
            T_sb = sb.tile([TB * 3 + 1, BPC], f32)
            nc.vector.tensor_copy(T_sb[:], psum_T[:])

            # G[k, tb, s] = -2 d0.shift + |shift|^2 (one matmul, N=216)
            psum_G = ps.tile([BPC, TB, 27], f32)
            nc.tensor.matmul(psum_G[:], T_sb[:], bd_sb[:], start=True, stop=True)

            Gmin = sb.tile([BPC, TB], f32)
            nc.vector.tensor_reduce(Gmin[:], psum_G[:], axis=AX.X, op=AL.min)
            bl2 = sb.tile([BPC, TB], f32)
            nc.vector.tensor_tensor(bl2[:], Gmin[:], d0sq[:], AL.add)
            bl2c = sb.tile([BPC, TB], f32)
            nc.vector.tensor_scalar_max(bl2c[:], bl2[:], 0.0)
            bl = sb.tile([BPC, TB], f32)
            nc.scalar.sqrt(bl[:], bl2c[:])
            nc.sync.dma_start(out_bl[:], bl[:])

    nc.compile()
    return nc


def _diag_box(cell):
    """Return box if cell == box*I (the layout setup_inputs produces), else None."""
    d = np.diag(cell)
    if (
        d[0] > 0
        and np.all(np.abs(d - d[0]) <= 1e-6 * abs(d[0]))
        and np.all(np.abs(cell - np.diag(d)) <= 1e-6 * abs(d[0]))
    ):
        return float(d[0])
    return None


def _get_nc(cell):
    box = _diag_box(cell)
    key = ("diag", box) if box is not None else ("general",)
    if key not in _NC_CACHE:
        _NC_CACHE[key] = (
            _build_nc_diag(box) if box is not None else _build_nc_general()
        )
    return _NC_CACHE[key], box


def _make_in_maps(radii, cell, bonds, need_bd):
    """Host-side sharding: per-core input dicts for the SPMD run."""
    import ml_dtypes

    bf16 = ml_dtypes.bfloat16
    arr = radii.reshape(TB, A, 128, 3).transpose(2, 1, 0, 3)  # [p, a, tb, c]
    hi = arr.astype(bf16)
    lo = (arr - hi.astype(np.float32)).astype(bf16)
    cs = np.ascontiguousarray(np.stack([hi, lo], axis=-1))  # [p, a, tb, c, 2]

    bd = None
    if need_bd:
        # Shift table from the tiny replicated constant `cell`.
        shifts = _combos_np() @ cell                      # [27, 3]
        ssq = (shifts * shifts).sum(axis=1)               # [27]
        bd = np.zeros((TB * 3 + 1, TB, 27), np.float32)
        for t in range(TB):
            bd[3 * t : 3 * t + 3, t, :] = -2.0 * shifts.T
        bd[TB * 3, :, :] = ssq[None, :]
        bd = np.ascontiguousarray(bd)

    in_maps = []
    for c in range(NCORES):
        sl = slice(c * BPC, (c + 1) * BPC)
        bb = np.broadcast_to(
            bonds[sl].T.astype(np.float16).reshape(1, 2, BPC), (128, 2, BPC)
        )
        m = {"coords": cs, "bb": np.ascontiguousarray(bb)}
        if need_bd:
            m["bd"] = bd
        in_maps.append(m)
    return in_maps


def kernel(**inputs):
    global LAST_RESULTS
    _ensure_import_path()
    from concourse.bass_utils import run_bass_kernel_spmd

    radii = np.asarray(inputs["stacked_radii"], dtype=np.float32)  # [T,B,N,3]
    cell = np.asarray(inputs["cell"], dtype=np.float32)
    bonds = np.asarray(inputs["bonds"]).astype(np.int64)           # [768,2]
    mbl = np.asarray(inputs["mean_bond_lens"], dtype=np.float32)   # [768]

    nc, box = _get_nc(cell)
    in_maps = _make_in_maps(radii, cell, bonds, need_bd=box is None)
    res = run_bass_kernel_spmd(nc, in_maps, core_ids=list(range(NCORES)))
    LAST_RESULTS = res

    # gather/unshard: assemble bond_lens; fold the deviation max over
    # (bonds, T) per replica at the same time (the hint's "final max over T").
    bond_lens = np.empty((T, B, NBONDS), np.float32)
    for c in range(NCORES):
        bond_lens[:, :, c * BPC : (c + 1) * BPC] = (
            res.results[c]["out_bl"].T.reshape(T, B, BPC)
        )
    dev = np.abs(bond_lens - mbl[None, None, :])
    max_dev = dev.max(axis=-1).max(axis=0).astype(np.float32)  # [B]
    return bond_lens, max_dev
